# revision 2
# baseline (speedup 1.0000x reference)
"""CoverageLoss kernel for 8 Trainium2 NeuronCores.

Algorithm (per the retrieval_knn structure):
  loss = size(ls) + size(la) + cov(ss, ls) + cov(sa, la)
  cov(S, L): d = cdist_l1(S, L); sm4 = 4 smallest per row; tail = sm4.mean(-1)
             far = top64(tail); loss = mean(sm4[far]**2)

Device strategy (2D shard grid: 4 latent-shards x 2 sample-shards):
  Phase A (TensorEngine): quantized-L1 via thermometer encoding.
    With per-coord threshold grid t_q and crossing weights w_q,
    Dq(s,l) = sum_eq w_q * |1[s_e>t_q] - 1[l_e>t_q]|.  Encoding the sample
    side as s' = (1[s>t] - 1/2) and latent side as l' = w*1[l>t] gives
    <s', l'> = const(s-row) - Dq/2, so per-row argmax-8 of the matmul output
    directly yields the 8 approximately-nearest latents (InstMax/MaxIndex).
  Phase B (exact): indirect-DMA gather of the 8 candidate latent rows per
    sample; exact fp32 L1 distances via tensor_reduce(abs).  Only these 8
    values per (sample, shard) feed the loss, so the result is fp32-exact
    up to candidate selection (calibrated: <1e-5 rel err on the loss).
  Size losses: per-row relu(|x|_1 - 1)^2 on device; host means.
Host: slice/replicate shards in, merge 4x8 exact candidates per sample,
  top-64 selection over 2048 tails, final scalar.
"""

import os
from contextlib import ExitStack

import numpy as np

import concourse.bass as bass
import concourse.bacc as bacc
import concourse.mybir as mybir
import concourse.tile as tile
from concourse.bass_utils import run_bass_kernel_spmd

# ---- quantizer constants (Gaussian-quantile thresholds, Q=16) ----
THR = [-2.142141580581665, -1.5157124996185303, -1.161488652229309,
       -0.8928850293159485, -0.6660019755363464, -0.4625875651836395,
       -0.27278241515159607, -0.09017427265644073, 0.09017427265644073,
       0.27278241515159607, 0.4625875651836395, 0.6660019755363464,
       0.8928850293159485, 1.161488652229309, 1.5157124996185303,
       2.142141580581665]
W = [0.31321457028388977, 0.490326464176178, 0.3114137649536133,
     0.24774335324764252, 0.21514873206615448, 0.19660978019237518,
     0.18620665371418, 0.181478351354599, 0.181478351354599,
     0.18620665371418, 0.19660978019237518, 0.21514873206615448,
     0.24774335324764252, 0.3114137649536133, 0.490326464176178,
     0.31321457028388977]

Q = 16
NLAT, ES, EA = 8192, 64, 32
NSMP = 2048
A_SHARDS, B_SHARDS = 4, 2          # latent shards x sample shards
NL = NLAT // A_SHARDS              # 2048 latents per core
MS = NSMP // B_SHARDS              # 1024 samples per core
NTILES = MS // 128                 # 8 sample tiles
KC_S = ES * Q // 128               # 8 k-chunks (states: 64 coords x 2 thr)
KC_A = EA * Q // 128               # 4 k-chunks (actions: 32 coords x 4 thr)
NCHUNK = NL // 512                 # 4 psum column chunks
NCAND = 8

F32 = mybir.dt.float32
F16 = mybir.dt.float16
U32 = mybir.dt.uint32


def _cov_kernel(ctx, tc, e, kc, latTx, smpTx, lat_rows, smp_rows,
                refc_out, tag):
    """Emit one coverage pipeline (states or actions).

    latTx: [128, NL+2kc] f32: (128/e)-replicated coord-major transpose of
           latents, then kc threshold cols, then kc weight cols (appended so
           each encode op depends on exactly one input DMA: the TS ISA
           encoding has a single sync-wait slot).
    smpTx: [128, MS+2kc] f32, same layout for samples
    lat_rows:  [NL, e] f32 DRAM gather table
    smp_rows:  [MS, e] f32 DRAM sample rows
    refc_out:  [MS, NCAND] f32 DRAM exact candidate distances
    """
    nc = tc.nc
    enc = ctx.enter_context(tc.tile_pool(name=f"enc_{tag}", bufs=kc))
    psum = ctx.enter_context(tc.tile_pool(name=f"psum_{tag}", bufs=4,
                                          space="PSUM"))
    work = ctx.enter_context(tc.tile_pool(name=f"work_{tag}", bufs=2))
    small = ctx.enter_context(tc.tile_pool(name=f"small_{tag}", bufs=4))

    # ---- encode: one tensor_scalar per k-chunk per side ----
    bl = []
    bs = []
    for c in range(kc):
        blc = enc.tile([128, NL], F16, tag=f"bl_{tag}")
        nc.vector.tensor_scalar(
            out=blc[:], in0=latTx[:, :NL], scalar1=latTx[:, NL + c:NL + c + 1],
            scalar2=latTx[:, NL + kc + c:NL + kc + c + 1],
            op0=mybir.AluOpType.is_gt, op1=mybir.AluOpType.mult)
        bl.append(blc)
        bsc = enc.tile([128, MS], F16, tag=f"bs_{tag}")
        nc.vector.tensor_scalar(
            out=bsc[:], in0=smpTx[:, :MS], scalar1=smpTx[:, MS + c:MS + c + 1],
            scalar2=0.5, op0=mybir.AluOpType.is_gt,
            op1=mybir.AluOpType.subtract)
        bs.append(bsc)

    # all sample rows in one strided DMA: smp_big[p, m*e + j] = smp[m*128+p, j]
    smp_big = small.tile([128, NTILES * e], F32, tag=f"smpbig_{tag}")
    nc.sync.dma_start(
        smp_big[:], smp_rows.rearrange("(m p) e -> p m e", p=128))

    # ---- per sample-tile: matmul -> top8 -> gather -> exact refine ----
    for m in range(NTILES):
        smp_tile = smp_big[:, m * e:(m + 1) * e]

        dneg = work.tile([128, NL], F32, tag=f"dneg_{tag}")
        for n in range(NCHUNK):
            ps = psum.tile([128, 512], F32, tag=f"ps_{tag}")
            for k in range(kc):
                nc.tensor.matmul(
                    ps[:], lhsT=bs[k][:, m * 128:(m + 1) * 128],
                    rhs=bl[k][:, n * 512:(n + 1) * 512],
                    start=(k == 0), stop=(k == kc - 1))
            nc.scalar.copy(dneg[:, n * 512:(n + 1) * 512], ps[:])

        max8 = small.tile([128, 8], F32, tag=f"max8_{tag}")
        idx8 = small.tile([128, 8], U32, tag=f"idx8_{tag}")
        nc.vector.max(out=max8[:], in_=dneg[:])
        nc.vector.max_index(out=idx8[:], in_max=max8[:], in_values=dneg[:])

        gath = work.tile([128, NCAND * e], F32, tag=f"gath_{tag}")
        nc.gpsimd.indirect_dma_start(
            out=gath[:], out_offset=None, in_=lat_rows[:, :],
            in_offset=bass.IndirectOffsetOnAxis(ap=idx8[:, :], axis=0))

        diff = work.tile([128, NCAND * e], F32, tag=f"diff_{tag}")
        g3 = gath[:].rearrange("p (c e) -> p c e", c=NCAND)
        s3 = smp_tile[:, None, :].broadcast_to([128, NCAND, e])
        d3 = diff[:].rearrange("p (c e) -> p c e", c=NCAND)
        nc.vector.tensor_tensor(out=d3, in0=g3, in1=s3,
                                op=mybir.AluOpType.subtract)
        refc = small.tile([128, NCAND], F32, tag=f"refc_{tag}")
        nc.vector.tensor_reduce(
            out=refc[:], in_=d3, axis=mybir.AxisListType.X,
            op=mybir.AluOpType.add, apply_absolute_value=True)
        nc.sync.dma_start(refc_out[m * 128:(m + 1) * 128, :], refc[:])


def _size_kernel(ctx, tc, e, lat_rows, sz_out, tag):
    """Per-row relu(|x|_1 - 1)^2 for a [NL, e] latent shard -> sz_out [128, NL//128]."""
    nc = tc.nc
    pool = ctx.enter_context(tc.tile_pool(name=f"sz_{tag}", bufs=1))
    nt = NL // 128
    lat_big = pool.tile([128, nt * e], F32, tag=f"latbig_{tag}")
    nc.sync.dma_start(
        lat_big[:], lat_rows.rearrange("(m p) e -> p m e", p=128))
    norms = pool.tile([128, nt], F32, tag=f"norms_{tag}")
    nc.vector.tensor_reduce(
        out=norms[:], in_=lat_big[:].rearrange("p (m e) -> p m e", m=nt),
        axis=mybir.AxisListType.X, op=mybir.AluOpType.add,
        apply_absolute_value=True)
    rl = pool.tile([128, nt], F32, tag=f"rl_{tag}")
    nc.vector.tensor_scalar(out=rl[:], in0=norms[:], scalar1=1.0, scalar2=0.0,
                            op0=mybir.AluOpType.subtract,
                            op1=mybir.AluOpType.max)
    sq = pool.tile([128, nt], F32, tag=f"sq_{tag}")
    nc.vector.tensor_tensor(out=sq[:], in0=rl[:], in1=rl[:],
                            op=mybir.AluOpType.mult)
    nc.sync.dma_start(sz_out[:, :], sq[:])


def _build_nc():
    nc = bacc.Bacc("TRN2", target_bir_lowering=False, debug=False,
                   num_devices=8)
    inp = {}
    for name, shape in [
        ("latT2_s", [128, NL + 2 * KC_S]), ("latT4_a", [128, NL + 2 * KC_A]),
        ("smpT2_s", [128, MS + 2 * KC_S]), ("smpT4_a", [128, MS + 2 * KC_A]),
        ("lat_s", [NL, ES]), ("lat_a", [NL, EA]),
        ("smp_s", [MS, ES]), ("smp_a", [MS, EA]),
    ]:
        inp[name] = nc.dram_tensor(name, shape, F32, kind="ExternalInput").ap()
    out = {}
    for name, shape in [
        ("refc_s", [MS, NCAND]), ("refc_a", [MS, NCAND]),
        ("szrows_s", [128, NL // 128]), ("szrows_a", [128, NL // 128]),
    ]:
        out[name] = nc.dram_tensor(name, shape, F32, kind="ExternalOutput").ap()

    with tile.TileContext(nc) as tc:
        with ExitStack() as ctx:
            big = ctx.enter_context(tc.tile_pool(name="bigin", bufs=1))
            tiles = {}
            for name in ("latT2_s", "latT4_a", "smpT2_s", "smpT4_a"):
                t = big.tile(list(inp[name].shape), F32, tag=name)
                nc.sync.dma_start(t[:], inp[name][:, :])
                tiles[name] = t

            _cov_kernel(ctx, tc, ES, KC_S, tiles["latT2_s"][:],
                        tiles["smpT2_s"][:], inp["lat_s"],
                        inp["smp_s"], out["refc_s"], "s")
            _cov_kernel(ctx, tc, EA, KC_A, tiles["latT4_a"][:],
                        tiles["smpT4_a"][:], inp["lat_a"],
                        inp["smp_a"], out["refc_a"], "a")
            _size_kernel(ctx, tc, ES, inp["lat_s"], out["szrows_s"], "s")
            _size_kernel(ctx, tc, EA, inp["lat_a"], out["szrows_a"], "a")
    nc.compile()
    return nc


_NC_CACHE = {}


def _get_nc():
    if "nc" not in _NC_CACHE:
        _NC_CACHE["nc"] = _build_nc()
    return _NC_CACHE["nc"]


def _make_in_maps(latent_states, latent_actions, state_space_samples,
                  action_space_samples):
    thr = np.asarray(THR, np.float32)
    w = np.asarray(W, np.float32)
    # chunk c, partition p: states -> (coord p%64, thr 2c + p//64)
    thr_s = np.stack([np.repeat(thr[2 * c:2 * c + 2], 64) for c in range(KC_S)], 1)
    w_s = np.stack([np.repeat(w[2 * c:2 * c + 2], 64) for c in range(KC_S)], 1)
    thr_a = np.stack([np.repeat(thr[4 * c:4 * c + 4], 32) for c in range(KC_A)], 1)
    w_a = np.stack([np.repeat(w[4 * c:4 * c + 4], 32) for c in range(KC_A)], 1)
    tw_s = np.concatenate([thr_s, w_s], 1)
    tw_a = np.concatenate([thr_a, w_a], 1)

    in_maps = []
    for core in range(8):
        a, b = core % A_SHARDS, core // A_SHARDS
        lat_s = np.ascontiguousarray(latent_states[a * NL:(a + 1) * NL])
        lat_a = np.ascontiguousarray(latent_actions[a * NL:(a + 1) * NL])
        smp_s = np.ascontiguousarray(state_space_samples[b * MS:(b + 1) * MS])
        smp_a = np.ascontiguousarray(action_space_samples[b * MS:(b + 1) * MS])
        in_maps.append({
            "latT2_s": np.ascontiguousarray(
                np.concatenate([np.tile(lat_s.T, (2, 1)), tw_s], 1)),
            "latT4_a": np.ascontiguousarray(
                np.concatenate([np.tile(lat_a.T, (4, 1)), tw_a], 1)),
            "smpT2_s": np.ascontiguousarray(
                np.concatenate([np.tile(smp_s.T, (2, 1)), tw_s], 1)),
            "smpT4_a": np.ascontiguousarray(
                np.concatenate([np.tile(smp_a.T, (4, 1)), tw_a], 1)),
            "lat_s": lat_s, "lat_a": lat_a, "smp_s": smp_s, "smp_a": smp_a,
        })
    return in_maps


def _host_combine(results):
    """results: list of 8 per-core output dicts -> final scalar loss."""
    total = np.float64(0)
    # size losses: states from b=0 cores, actions from b=1 cores
    sz_s = [results[a]["szrows_s"] for a in range(A_SHARDS)]
    sz_a = [results[A_SHARDS + a]["szrows_a"] for a in range(A_SHARDS)]
    total += np.concatenate([s.ravel() for s in sz_s]).mean(dtype=np.float64)
    total += np.concatenate([s.ravel() for s in sz_a]).mean(dtype=np.float64)
    # coverage: merge per-shard exact candidate distances
    for key in ("refc_s", "refc_a"):
        ref = np.empty((NSMP, A_SHARDS * NCAND), np.float32)
        for core in range(8):
            a, b = core % A_SHARDS, core // A_SHARDS
            ref[b * MS:(b + 1) * MS, a * NCAND:(a + 1) * NCAND] = \
                results[core][key]
        ref.sort(axis=-1)
        sm4 = ref[:, :4]
        tails = sm4.mean(-1)
        far = np.argsort(-tails)[:64]
        total += np.float64((sm4[far].astype(np.float64) ** 2).mean())
    return np.float32(total)


def kernel(latent_states, latent_actions, state_space_samples,
           action_space_samples, _want_results=False, _trace=False,
           _tmpdir=None):
    nc = _get_nc()
    in_maps = _make_in_maps(latent_states, latent_actions,
                            state_space_samples, action_space_samples)
    res = run_bass_kernel_spmd(nc, in_maps, core_ids=list(range(8)),
                               trace=_trace, tmpdir=_tmpdir)
    out = _host_combine(res.results)
    if _want_results:
        return out, res
    return out



# revision 4
# speedup vs baseline: 2.5558x; 2.5558x over previous
"""CoverageLoss kernel for 8 Trainium2 NeuronCores.

Redesign vs the thermometer-quantized-L1 kernel (150us):
  Candidate metric is squared-L2 via one bf16 K=e+1 matmul per 512-latent
  chunk (score = <s,l> - |l|^2/2, monotone in -L2^2 per row) instead of a
  Q=16 thermometer (K=1024) -- 16x less contraction and bf16-rate
  streaming (fp32 moving operands measured ~5x slower).  Per-row top-k
  runs on a group-max-coarsened score row: reduce groups of G=32 latents
  to their max, then MAX8/FIND_INDEX8 scan only [128, 64] group maxima
  (MAX8/FI8 have no fast DVE perf mode, so shrinking their input 32x is
  the only lever).  Top-8-of-group-maxima covers the true top-8 elements
  (element #k's group ranks <= k).  The group reduce is split across
  engines per PATHS: 'R' tiles use DVE tensor_reduce straight from PSUM
  (1x, but drains PSUM without ScalarE); 'V' tiles use a ScalarE
  psum->bf16 convert plus a DVE halving cascade of tensor_tensor(max)
  (2x_1p) -- groups come out as stride-NGRP lattices {g + NGRP*j} either
  way.  Host expands the winning groups and refines exact L1 there, so
  the device ships only [MS, 8] values+indices per (problem, shard).

Sharding: 4 latent-shards x 2 sample-shards, latents carry a -|l|^2/2 row;
samples carry a ones row.  Host: group expansion, exact L1 refine of global
top-TOP_T groups, tails, top-64 far selection, size losses, final scalar.
"""

from contextlib import ExitStack

import numpy as np

import concourse.bacc as bacc
import concourse.mybir as mybir
import concourse.tile as tile
from concourse.bass_utils import run_bass_kernel_spmd

NLAT, ES, EA = 8192, 64, 32
NSMP = 2048
A_SHARDS, B_SHARDS = 4, 2
NL = NLAT // A_SHARDS              # 2048 latents per core
MS = NSMP // B_SHARDS              # 1024 samples per core
NTILES = MS // 128                 # 8 sample tiles per problem
NCHUNK = NL // 512                 # 4 psum column chunks
G = 32                             # latents per group (stride-NGRP lattice)
NGRP = NL // G                     # 64 groups per shard
TOP_T = 8                          # global top groups refined on host

F32 = mybir.dt.float32
BF16 = mybir.dt.bfloat16
U16 = mybir.dt.uint16

# Per-tile engine routing for the group-max reduce (16 tiles, s/a
# interleaved).  'R': DVE tensor_reduce straight from PSUM (1x but drains
# PSUM without ScalarE).  'V': ScalarE psum->bf16 convert + DVE 3-fold
# tensor_tensor(max) cascade (2x_1p).  'P': ScalarE convert + GPSIMD folds.
# Groups are stride-NGRP lattices {g + NGRP*j} for every path.
PATHS = ['V', 'V', 'V', 'V', 'V', 'R', 'V', 'V',
         'V', 'V', 'V', 'R', 'V', 'V', 'V', 'V']


def _build_nc():
    nc = bacc.Bacc("TRN2", target_bir_lowering=False, debug=False,
                   num_devices=8)
    inp = {}
    for name, shape in [
        ("latK_s", [ES + 1, NL]), ("latK_a", [EA + 1, NL]),
        ("smpK_s", [ES + 1, MS]), ("smpK_a", [EA + 1, MS]),
    ]:
        inp[name] = nc.dram_tensor(name, shape, BF16,
                                   kind="ExternalInput").ap()
    out = {}
    for name, shape, dt in [
        ("gv_s", [MS, 8], F32), ("gi_s", [MS, 8], U16),
        ("gv_a", [MS, 8], F32), ("gi_a", [MS, 8], U16),
    ]:
        out[name] = nc.dram_tensor(name, shape, dt, kind="ExternalOutput").ap()

    with tile.TileContext(nc) as tc, ExitStack() as ctx:
        ins = ctx.enter_context(tc.tile_pool(name="ins", bufs=1))
        sb = ctx.enter_context(tc.tile_pool(name="sb", bufs=3))
        fold = ctx.enter_context(tc.tile_pool(name="fold", bufs=4))
        red = ctx.enter_context(tc.tile_pool(name="red", bufs=4))
        out8 = ctx.enter_context(tc.tile_pool(name="out8", bufs=6))
        ps_pool = {
            "s": ctx.enter_context(tc.tile_pool(name="ps_s", bufs=1,
                                                space="PSUM")),
            "a": ctx.enter_context(tc.tile_pool(name="ps_a", bufs=1,
                                                space="PSUM")),
        }
        prob = {}
        for tag, e in (("s", ES), ("a", EA)):
            K = e + 1
            latT = ins.tile([K, NL], BF16, tag=f"latT_{tag}")
            tc.nc.sync.dma_start(latT[:], inp[f"latK_{tag}"][:, :])
            smpT = ins.tile([K, MS], BF16, tag=f"smpT_{tag}")
            tc.nc.sync.dma_start(smpT[:], inp[f"smpK_{tag}"][:, :])
            prob[tag] = (latT, smpT)

        gidx = 0
        for m in range(NTILES):
            for tag in ("s", "a"):
                latT, smpT = prob[tag]
                nc_ = tc.nc
                ps = ps_pool[tag].tile([128, NL], F32, tag=f"ps_{tag}")
                for n in range(NCHUNK):
                    nc_.tensor.matmul(
                        ps[:, n * 512:(n + 1) * 512],
                        lhsT=smpT[:, m * 128:(m + 1) * 128],
                        rhs=latT[:, n * 512:(n + 1) * 512],
                        start=True, stop=True)

                rg = red.tile([128, NGRP], BF16, tag=f"rg_{tag}")
                path = PATHS[gidx]
                if path == 'R':
                    nc_.vector.tensor_reduce(
                        out=rg[:],
                        in_=ps[:].rearrange("p (k g) -> p g k", k=G),
                        axis=mybir.AxisListType.X, op=mybir.AluOpType.max)
                else:
                    sbt = sb.tile([128, NL], BF16, tag=f"sb_{tag}")
                    nc_.scalar.copy(sbt[:], ps[:])
                    cur = sbt
                    w = NL
                    while w > 2 * NGRP:
                        w //= 2
                        nxt = fold.tile([128, w], BF16, tag=f"f{w}_{tag}")
                        nc_.vector.tensor_tensor(
                            out=nxt[:], in0=cur[:, :w], in1=cur[:, w:2 * w],
                            op=mybir.AluOpType.max)
                        cur = nxt
                    nc_.vector.tensor_tensor(
                        out=rg[:], in0=cur[:, :NGRP], in1=cur[:, NGRP:],
                        op=mybir.AluOpType.max)

                v8 = out8.tile([128, 8], F32, tag=f"v8_{tag}")
                i8 = out8.tile([128, 8], U16, tag=f"i8_{tag}")
                nc_.vector.max(out=v8[:], in_=rg[:])
                nc_.vector.max_index(out=i8[:], in_max=v8[:], in_values=rg[:])
                nc_.sync.dma_start(
                    out[f"gv_{tag}"][m * 128:(m + 1) * 128, :], v8[:])
                nc_.sync.dma_start(
                    out[f"gi_{tag}"][m * 128:(m + 1) * 128, :], i8[:])
                gidx += 1
    nc.compile()
    return nc


_NC_CACHE = {}


def _get_nc():
    if "nc" not in _NC_CACHE:
        _NC_CACHE["nc"] = _build_nc()
    return _NC_CACHE["nc"]


import ml_dtypes

BF16_NP = ml_dtypes.bfloat16


def _prep(lat, smp):
    """Build [e+1, NL] latent and [e+1, MS] sample operands (bf16)."""
    latK = np.concatenate(
        [lat.T, -0.5 * (lat.astype(np.float64) ** 2).sum(
            -1, keepdims=True).T.astype(np.float32)], axis=0)
    smpK = np.concatenate(
        [smp.T, np.ones((1, smp.shape[0]), np.float32)], axis=0)
    return (np.ascontiguousarray(latK.astype(BF16_NP)),
            np.ascontiguousarray(smpK.astype(BF16_NP)))


def _make_in_maps(latent_states, latent_actions, state_space_samples,
                  action_space_samples):
    in_maps = []
    for core in range(8):
        a, b = core % A_SHARDS, core // A_SHARDS
        latK_s, smpK_s = _prep(latent_states[a * NL:(a + 1) * NL],
                               state_space_samples[b * MS:(b + 1) * MS])
        latK_a, smpK_a = _prep(latent_actions[a * NL:(a + 1) * NL],
                               action_space_samples[b * MS:(b + 1) * MS])
        in_maps.append({"latK_s": latK_s, "smpK_s": smpK_s,
                        "latK_a": latK_a, "smpK_a": smpK_a})
    return in_maps


def _size_loss(lat):
    norms = np.abs(lat.astype(np.float64)).sum(-1)
    viol = np.maximum(norms - 1.0, 0.0)
    return (viol ** 2).mean()


def _cov_host(results, lat_full, smp_full, key_v, key_i):
    """Merge per-core group candidates -> exact L1 -> coverage loss term."""
    S = NSMP
    vals = np.empty((S, A_SHARDS * 8), np.float32)
    gids = np.empty((S, A_SHARDS * 8), np.int64)
    for core in range(8):
        a, b = core % A_SHARDS, core // A_SHARDS
        vals[b * MS:(b + 1) * MS, a * 8:(a + 1) * 8] = \
            results[core][key_v].astype(np.float32)
        gids[b * MS:(b + 1) * MS, a * 8:(a + 1) * 8] = \
            results[core][key_i].astype(np.int64) + a * NL
    sel = np.argsort(-vals, axis=1)[:, :TOP_T]
    gsel = np.take_along_axis(gids, sel, axis=1)            # [S, T]
    # group g of shard a (base latent a*NL+g) holds latents {base + NGRP*j}
    cand = (gsel[:, :, None] +
            (NGRP * np.arange(G))[None, None, :]).reshape(S, TOP_T * G)
    tails = np.empty(S)
    sm4s = np.empty((S, 4))
    lat32 = np.ascontiguousarray(lat_full, np.float32)
    smp32 = np.ascontiguousarray(smp_full, np.float32)
    for i in range(0, S, 512):
        gl = lat32[cand[i:i + 512]]                         # [c, T*G, e]
        d = np.abs(smp32[i:i + 512, None, :] - gl).sum(-1, dtype=np.float64)
        part = np.partition(d, 3, axis=1)[:, :4]
        sm4s[i:i + 512] = np.sort(part, axis=1)
        tails[i:i + 512] = part.mean(-1)
    far = np.argsort(-tails)[:64]
    return (sm4s[far] ** 2).mean()


def kernel(latent_states, latent_actions, state_space_samples,
           action_space_samples, _want_results=False, _trace=False,
           _tmpdir=None):
    nc = _get_nc()
    in_maps = _make_in_maps(latent_states, latent_actions,
                            state_space_samples, action_space_samples)
    res = run_bass_kernel_spmd(nc, in_maps, core_ids=list(range(8)),
                               trace=_trace, tmpdir=_tmpdir)
    total = np.float64(0)
    total += _size_loss(latent_states) + _size_loss(latent_actions)
    total += _cov_host(res.results, latent_states, state_space_samples,
                       "gv_s", "gi_s")
    total += _cov_host(res.results, latent_actions, action_space_samples,
                       "gv_a", "gi_a")
    out = np.float32(total)
    if _want_results:
        return out, res
    return out
